# revision 1
# baseline (speedup 1.0000x reference)
"""AttentionBlock (1x1-conv QKV + 4-head softmax attention + 1x1-conv proj)
on 8 Trainium2 NeuronCores.

Sharding: data-parallel over (batch b, query-half h) -> 8 shards. Each core
gets x rotated so its 2048 query columns are always columns 0:2048 (key order
is a permutation, which softmax-attention is invariant to), computes
qkv projections, 4-head attention for its half of the queries, and the output
projection for its [256, 2048] output slice. No collectives.

Core kernel tricks:
  - all matmuls in float32r (full-rate PE, ~1.5e-4 rel rounding)
  - S^T = K^T Q with two heads row-tiled in the PE array (K=64 each)
  - exp of scores: half the heads on the Scalar engine (exact), half via a
    fused custom DVE op ((x+c0)((x+c1)x+c2))^8 ~ C*e^x (scale cancels in
    softmax; assignment is per-(head, query-tile) so rows stay consistent)
  - attn @ V with two heads col-tiled, plus 4-way col-tiled ones-matmul
    rowsums accumulated in PSUM
  - softmax normalization via reciprocal + a tiny broadcast matmul
"""
import sys

sys.path.insert(0, '/opt/trn_rl_repo')

import numpy as np
from contextlib import ExitStack

from concourse import bass, bacc, mybir
import concourse.tile as tile
from concourse import dve_ops
from concourse.dve_ops import DveOp, OPS, CUSTOM_DVE_SPECS, _SUB_OPCODE_FOR_NAME
from concourse.dve_spec import Spec, Src0, Src1, C0, C1, C2, C3, lower, sq, _spill_c3_to_src1
from concourse.dve_uop import DveOpSpec
from concourse.bass_utils import run_bass_kernel_spmd

F32 = mybir.dt.float32
F32R = mybir.dt.float32r
BF16 = mybir.dt.bfloat16
ActFn = mybir.ActivationFunctionType

B, C, H, W = 4, 256, 64, 64
HEADS, DH = 4, 64
N = H * W            # 4096 keys
NQ = N // 2          # 2048 queries per core
NT = 512             # query tile (one PSUM bank of fp32)
N_NT = NQ // NT      # 4 query tiles
N_MC = N // 128      # 32 key chunks

# exp(x) ~ C * [q3(x) * (x^2 + b0 x + b1)]^16 over x in [-8.8, 8.4]
# (max rel err 3.3e-4; the constant C cancels in softmax normalization).
# Two DVE instructions: EXP5A computes the cubic q3, EXP5B multiplies by the
# monic quadratic and raises to the 16th power.
EXP_A = (0.00039684202121525346, 2.589769573122113e-05,
         6.891462469732395e-07, 7.771052073346383e-09)   # a0..a3
EXP_B = (-6.95331830849084, 2519.7822812996437)          # b0, b1


def _ref_exp5a(in0, in1, c0, c1, c2):
    x = in0.astype(np.float32)
    a3 = in1.astype(np.float32) if isinstance(in1, np.ndarray) else np.float32(in1)
    return (((a3 * x + np.float32(c2)) * x + np.float32(c1)) * x
            + np.float32(c0)).astype(np.float32)


def _ref_exp5b(in0, in1, c0, c1, c2):
    x = in0.astype(np.float32)
    q3 = in1.astype(np.float32)
    p = (q3 * ((x + np.float32(c0)) * x + np.float32(c1))).astype(np.float32)
    for _ in range(4):
        p = (p * p).astype(np.float32)
    return p


def _register(name, spec, rd1_en):
    row = dve_ops._CUSTOM_DVE_ROW_BASE + len(OPS)
    assert row < 0x20
    _SUB_OPCODE_FOR_NAME[name] = row
    shas = {}
    for ver in ("v3", "v4"):
        uops = lower(spec, ver=ver)
        shas[ver] = DveOpSpec(name=name, opcode=row, uops=uops, rd1_en=rd1_en).sha(ver)
    op = DveOp(name, spec, subdim=False, uops_sha=shas)
    OPS.append(op)
    CUSTOM_DVE_SPECS[name] = spec
    return op


def register_exp_op():
    if "EXP5A_ANT" in _SUB_OPCODE_FOR_NAME:
        a = next(op for op in OPS if op.name == "EXP5A_ANT")
        b = next(op for op in OPS if op.name == "EXP5B_ANT")
        return a, b
    x = Src0
    body_a = _spill_c3_to_src1(((C3 * x + C2) * x + C1) * x + C0)
    op_a = _register("EXP5A_ANT", Spec(body=body_a, reference=_ref_exp5a), True)
    body_b = sq(sq(sq(sq(Src1 * ((x + C0) * x + C1)))))
    op_b = _register("EXP5B_ANT", Spec(body=body_b, reference=_ref_exp5b), True)
    return op_a, op_b


def emit_exp_dve(nc, ops, out, in_, y1, a3_t):
    op_a, op_b = ops
    nc.vector._custom_dve(op_a, out=y1, in0=in_, in1=a3_t,
                          s0=float(EXP_A[0]), s1=float(EXP_A[1]), imm2=float(EXP_A[2]))
    return nc.vector._custom_dve(op_b, out=out, in0=in_, in1=y1,
                                 s0=float(EXP_B[0]), s1=float(EXP_B[1]))


# exp-engine split: ACT computes pair-0 tiles fully plus the first EXP_N0
# query-columns of each pair-1 head; the DVE two-op pipeline takes the rest.
# Constant per (pair, nt, n-range) so every softmax row uses one implementation.
import os as _os
EXP_N0 = int(_os.environ.get("EXP_N0", "192"))


def build_program(exp_op):
    nc = bacc.Bacc(target_bir_lowering=False)

    x_d = nc.declare_dram_parameter("x", [C, N], F32, isOutput=False)
    wq_d = nc.declare_dram_parameter("wq", [C, C], F32, isOutput=False)
    wk_d = nc.declare_dram_parameter("wk", [C, C], F32, isOutput=False)
    wv_d = nc.declare_dram_parameter("wv", [C, C], F32, isOutput=False)
    wp_d = nc.declare_dram_parameter("wp", [C, C], F32, isOutput=False)
    bias_d = nc.declare_dram_parameter("bias", [128, 2], F32, isOutput=False)
    y_d = nc.declare_dram_parameter("y", [C, NQ], F32, isOutput=True)
    import os as _os
    _DBG = bool(int(_os.environ.get("KERNEL_DEBUG", "0")))
    dbg = {}
    if _DBG:
        BF16_ = mybir.dt.bfloat16
        dbg["q0"] = nc.declare_dram_parameter("dbg_q0", [128, 512], F32, isOutput=True)
        dbg["st0"] = nc.declare_dram_parameter("dbg_st0", [128, 1024], F32, isOutput=True)
        dbg["e0"] = nc.declare_dram_parameter("dbg_e0", [128, 1024], BF16_, isOutput=True)
        dbg["e1"] = nc.declare_dram_parameter("dbg_e1", [128, 1024], BF16_, isOutput=True)
        dbg["rs"] = nc.declare_dram_parameter("dbg_rs", [128, 512], F32, isOutput=True)
        dbg["rsinv"] = nc.declare_dram_parameter("dbg_rsinv", [128, 512], F32, isOutput=True)
        dbg["rb0"] = nc.declare_dram_parameter("dbg_rb0", [128, 512], F32, isOutput=True)
        dbg["rb1"] = nc.declare_dram_parameter("dbg_rb1", [128, 512], F32, isOutput=True)
        dbg["po0"] = nc.declare_dram_parameter("dbg_po0", [128, 512], F32, isOutput=True)
        dbg["vt0"] = nc.declare_dram_parameter("dbg_vt0", [128, 256], BF16_, isOutput=True)

    with tile.TileContext(nc) as tc, ExitStack() as ctx:
        sb = ctx.enter_context(tc.tile_pool(name="sb", bufs=1))
        pex = ctx.enter_context(tc.tile_pool(name="pex", bufs=3))
        pout = ctx.enter_context(tc.tile_pool(name="pout", bufs=2))
        ps = ctx.enter_context(tc.tile_pool(name="ps", bufs=1, space="PSUM"))

        # ---------------- load + round inputs to f32r ----------------
        x_f = [sb.tile([128, N], F32, tag=f"xf{i}", name=f"xf{i}") for i in range(2)]
        x_r = [sb.tile([128, N], F32R, tag=f"xr{i}", name=f"xr{i}") for i in range(2)]
        for kc in range(2):
            nc.sync.dma_start(out=x_f[kc], in_=x_d[kc * 128:(kc + 1) * 128, :])
        nc.scalar.copy(x_r[0][:, :], x_f[0][:, :])
        nc.vector.tensor_copy(x_r[1][:, :], x_f[1][:, :])

        w_sb = {}
        for name, dram in (("wq", wq_d), ("wk", wk_d), ("wv", wv_d), ("wp", wp_d)):
            tiles = []
            for kc in range(2):
                f = sb.tile([128, C], F32, tag="wf", name=f"{name}f{kc}")
                nc.sync.dma_start(out=f, in_=dram[kc * 128:(kc + 1) * 128, :])
                r = sb.tile([128, C], F32R, tag=f"{name}{kc}", name=f"{name}r{kc}")
                (nc.vector.tensor_copy if kc else nc.scalar.copy)(r[:, :], f[:, :])
                tiles.append(r)
            w_sb[name] = tiles
        bias_sb = sb.tile([128, 2], F32, tag="bias")
        nc.sync.dma_start(out=bias_sb, in_=bias_d[:, :])

        # constants: ones column + broadcast matrices
        ones = sb.tile([128, 1], BF16, tag="ones")
        nc.vector.memset(ones, 1.0)
        a3_t = sb.tile([128, 1], F32, tag="a3")
        nc.vector.memset(a3_t, float(EXP_A[3]))
        zero_f = sb.tile([128, 512], F32, tag="zerof")
        nc.vector.memset(zero_f, 0.0)
        # broadcast matrix: out[m, n] = rhs[32*head(m) + 64*oc, n]
        bc_f = sb.tile([128, 256], F32, tag="bc_f")
        nc.vector.memset(bc_f, 0.0)
        nc.vector.memset(bc_f[0:1, 0:64], 1.0)
        nc.vector.memset(bc_f[32:33, 64:128], 1.0)
        nc.vector.memset(bc_f[64:65, 128:192], 1.0)
        nc.vector.memset(bc_f[96:97, 192:256], 1.0)
        bc = sb.tile([128, 256], F32R, tag="bc")
        nc.vector.tensor_copy(bc, bc_f[:, :])


        # ---------------- phase 1: qkv projections ----------------
        q_sb = [sb.tile([128, NQ], F32R, tag=f"q{oc}", name=f"q_sb{oc}") for oc in range(2)]
        k_sb = [sb.tile([128, N], F32R, tag=f"k{oc}", name=f"k_sb{oc}") for oc in range(2)]
        vT_sb = sb.tile([128, N_MC * 256], BF16, tag="vT")

        for oc in range(2):
            for nt in range(N_NT):
                pq = ps.tile([128, 512], F32, tag="s", bufs=2, name=f"pq{oc}_{nt}")
                sl = slice(nt * 512, (nt + 1) * 512)
                nc.tensor.matmul(out=pq[:, :], lhsT=w_sb["wq"][0][:, oc * 128:(oc + 1) * 128],
                                 rhs=x_r[0][:, sl], start=True, stop=False)
                nc.tensor.matmul(out=pq[:, :], lhsT=w_sb["wq"][1][:, oc * 128:(oc + 1) * 128],
                                 rhs=x_r[1][:, sl], start=False, stop=True)
                (nc.scalar.copy if (oc + nt) % 2 else nc.vector.tensor_copy)(q_sb[oc][:, sl], pq[:, :])
                if _DBG and oc == 0 and nt == 0:
                    nc.sync.dma_start(out=dbg["q0"][:, :], in_=q_sb[0][:, 0:512].bitcast(F32))
        for oc in range(2):
            for nt in range(2 * N_NT):
                pk = ps.tile([128, 512], F32, tag="s", bufs=2, name=f"pk{oc}_{nt}")
                sl = slice(nt * 512, (nt + 1) * 512)
                nc.tensor.matmul(out=pk[:, :], lhsT=w_sb["wk"][0][:, oc * 128:(oc + 1) * 128],
                                 rhs=x_r[0][:, sl], start=True, stop=False)
                nc.tensor.matmul(out=pk[:, :], lhsT=w_sb["wk"][1][:, oc * 128:(oc + 1) * 128],
                                 rhs=x_r[1][:, sl], start=False, stop=True)
                (nc.vector.tensor_copy if nt % 2 else nc.scalar.copy)(k_sb[oc][:, sl], pk[:, :])
        for mc in range(N_MC):
            pv = ps.tile([128, 256], F32, tag="rs", name=f"pv{mc}")
            msl = slice(mc * 128, (mc + 1) * 128)
            nc.tensor.matmul(out=pv[:, :], lhsT=x_r[0][:, msl], rhs=w_sb["wv"][0][:, :],
                             start=True, stop=False)
            nc.tensor.matmul(out=pv[:, :], lhsT=x_r[1][:, msl], rhs=w_sb["wv"][1][:, :],
                             start=False, stop=True)
            (nc.vector.tensor_copy if mc % 2 else nc.scalar.copy)(
                vT_sb[:, mc * 256:(mc + 1) * 256], pv[:, :])
            if _DBG and mc == 0:
                nc.sync.dma_start(out=dbg["vt0"][:, :], in_=vT_sb[:, 0:256])

        import os
        _PH = int(os.environ.get("KERNEL_PHASES", "3"))
        # ---------------- phase 2: attention ----------------
        out_sp = [sb.tile([128, NQ], F32R, tag=f"osp{oc}", name=f"out_sp{oc}") for oc in range(2)]

        if _PH < 2:
            zero_f = sb.tile([128, 512], F32, tag="zero_f")
            nc.vector.memset(zero_f, 0.0)
            for oc in range(2):
                for z in range(4):
                    nc.vector.tensor_copy(out_sp[oc][:, z * 512:(z + 1) * 512], zero_f[:, :])
        for nt in range(N_NT if _PH >= 2 else 0):
            qsl = slice(nt * 512, (nt + 1) * 512)
            po = [ps.tile([128, 512], F32, tag="o", bufs=3, name=f"po{pair}_{nt}") for pair in (0, 1)]
            prs = ps.tile([128, 512], F32, tag="rs", name=f"prs{nt}")
            for mc in range(N_MC):
                msl = slice(mc * 128, (mc + 1) * 128)
                exps = []
                for pair in (0, 1):
                    pst = ps.tile([128, 1024], F32, tag="s", bufs=2, name=f"pst{pair}_{nt}_{mc}")
                    # S^T: two heads row-tiled (dh=64 each)
                    nc.tensor.matmul(out=pst[:, 0:512],
                                     lhsT=k_sb[pair][0:64, msl], rhs=q_sb[pair][0:64, qsl],
                                     start=True, stop=True, tile_position=(0, 0))
                    nc.tensor.matmul(out=pst[:, 512:1024],
                                     lhsT=k_sb[pair][64:128, msl], rhs=q_sb[pair][64:128, qsl],
                                     start=True, stop=True, tile_position=(64, 0))
                    et = pex.tile([128, 1024], BF16, tag=f"e{pair}", name=f"et{pair}_{nt}_{mc}")
                    if pair == 0 or EXP_N0 >= 512:
                        nc.scalar.activation(et[:, :], pst[:, :], ActFn.Exp)
                    elif EXP_N0 == 0:
                        y1 = pex.tile([128, 1024], F32, tag="y1", name=f"y1_{pair}_{nt}_{mc}")
                        emit_exp_dve(nc, exp_op, et[:, :], pst[:, :], y1[:, :], a3_t[:, :])
                    else:
                        # strided APs covering (h2 cols [a:b]) u (h3 cols [512+a:512+b])
                        def _two(ap_t, a, b):
                            base = ap_t[:, a:b]
                            return bass.AP(tensor=base.tensor, offset=base.offset,
                                           ap=[list(base.ap[0]), [512, 2], [1, b - a]])
                        nc.scalar.activation(_two(et, 0, EXP_N0), _two(pst, 0, EXP_N0),
                                             ActFn.Exp)
                        y1 = pex.tile([128, 1024], F32, tag="y1", name=f"y1_{pair}_{nt}_{mc}")
                        emit_exp_dve(nc, exp_op, _two(et, EXP_N0, 512),
                                     _two(pst, EXP_N0, 512), _two(y1, EXP_N0, 512),
                                     a3_t[:, :])
                    if _DBG and nt == 0 and mc == 0:
                        nc.sync.dma_start(out=dbg[f"e{pair}"][:, :], in_=et[:, :])
                        if pair == 0:
                            st_f = sb.tile([128, 1024], F32, tag="dbg_st", name="dbg_st_t")
                            nc.vector.tensor_copy(st_f, pst[:, :])
                            nc.sync.dma_start(out=dbg["st0"][:, :], in_=st_f)
                    exps.append(et)
                first, last = mc == 0, mc == N_MC - 1
                for pair in (0, 1):
                    vb = mc * 256 + pair * 128
                    nc.tensor.matmul(out=po[pair][0:64, :],
                                     lhsT=vT_sb[:, vb:vb + 64], rhs=exps[pair][:, 0:512],
                                     start=first, stop=last, tile_position=(0, 0))
                    nc.tensor.matmul(out=po[pair][64:128, :],
                                     lhsT=vT_sb[:, vb + 64:vb + 128], rhs=exps[pair][:, 512:1024],
                                     start=first, stop=last, tile_position=(0, 64))
                for hh in range(4):
                    nc.tensor.matmul(out=prs[32 * hh:32 * hh + 1, :],
                                     lhsT=ones[:, :], rhs=exps[hh // 2][:, (hh % 2) * 512:(hh % 2 + 1) * 512],
                                     start=first, stop=last, tile_position=(0, 32 * hh))
            if _DBG and nt == 0:
                rs_f = sb.tile([128, 512], F32, tag="dbg_rs", name="dbg_rs_t")
                nc.vector.tensor_copy(rs_f[0:1, :], prs[0:1, :])
                nc.vector.tensor_copy(rs_f[32:33, :], prs[32:33, :])
                nc.vector.tensor_copy(rs_f[64:65, :], prs[64:65, :])
                nc.vector.tensor_copy(rs_f[96:97, :], prs[96:97, :])
                nc.sync.dma_start(out=dbg["rs"][:, :], in_=rs_f)
                po_f = sb.tile([128, 512], F32, tag="dbg_po", name="dbg_po_t")
                nc.vector.tensor_copy(po_f, po[1][:, :])
                nc.sync.dma_start(out=dbg["po0"][:, :], in_=po_f)
            # normalization: copy the 4 rowsum rows to SBUF (ACT, f32r), matmul
            # against the 0/1 broadcast matrix to replicate each head's rowsum
            # to its 64 output partitions, evacuate to SBUF, reciprocal, multiply.
            # (custom DVE ops and partition_broadcast only work at base 0.)
            rs_sb = sb.tile([128, 512], F32R, tag="rs_sb", name=f"rs_sb{nt}")
            # zero-fill: the broadcast matmul reads all 128 partitions and
            # uninitialized SBUF can contain NaNs (0 * NaN = NaN)
            nc.vector.tensor_copy(rs_sb[:, :], zero_f[:, :])
            for hh in range(4):
                nc.scalar.copy(rs_sb[32 * hh:32 * hh + 1, :], prs[32 * hh:32 * hh + 1, :])
            for oc in range(2):
                pb = ps.tile([128, 512], F32, tag="s", bufs=2, name=f"pb{oc}_{nt}")
                nc.tensor.matmul(out=pb[:, :], lhsT=bc[:, oc * 128:(oc + 1) * 128],
                                 rhs=rs_sb[:, :], start=True, stop=True)
                rbr = sb.tile([128, 512], F32, tag="rbr", name=f"rbr{oc}_{nt}")
                nc.scalar.copy(rbr[:, :], pb[:, :])
                rb = sb.tile([128, 512], F32, tag="rb", name=f"rb{oc}_{nt}")
                nc.vector.reciprocal_approx_fast(out=rb[:, :], in_=rbr[:, :])
                nc.vector.tensor_tensor(
                    out=out_sp[oc][:, qsl], in0=po[oc][:, :], in1=rb[:, :],
                    op=mybir.AluOpType.mult)
                if _DBG and nt == 0:
                    nc.sync.dma_start(out=dbg[f"rb{oc}"][:, :], in_=rb[:, :])
            if _DBG and nt == 0:
                nc.sync.dma_start(out=dbg["rsinv"][:, :], in_=rs_sb[:, :])

        # ---------------- phase 3: output projection + bias ----------------
        for oc in range(2):
            for nt in range(N_NT):
                sl = slice(nt * 512, (nt + 1) * 512)
                py = ps.tile([128, 512], F32, tag="o", bufs=3, name=f"py{oc}_{nt}")
                nc.tensor.matmul(out=py[:, :], lhsT=w_sb["wp"][0][:, oc * 128:(oc + 1) * 128],
                                 rhs=out_sp[0][:, sl], start=True, stop=False)
                nc.tensor.matmul(out=py[:, :], lhsT=w_sb["wp"][1][:, oc * 128:(oc + 1) * 128],
                                 rhs=out_sp[1][:, sl], start=False, stop=True)
                y_sb = pout.tile([128, 512], F32, tag="y", name=f"y_sb{oc}_{nt}")
                nc.vector.tensor_scalar_add(y_sb[:, :], py[:, :], bias_sb[:, oc:oc + 1])
                nc.sync.dma_start(out=y_d[oc * 128:(oc + 1) * 128, sl], in_=y_sb[:, :])

    nc.compile()
    return nc


_CACHE = {}


def _get_program():
    if "nc" not in _CACHE:
        op = register_exp_op()
        _CACHE["nc"] = build_program(op)
    return _CACHE["nc"]


def kernel(x, w_qkv, w_proj, b_proj):
    x = np.asarray(x, np.float32)
    w_qkv = np.asarray(w_qkv, np.float32)
    w_proj = np.asarray(w_proj, np.float32)
    b_proj = np.asarray(b_proj, np.float32)

    nc = _get_program()

    x2 = x.reshape(B, C, N)
    wq_t = np.ascontiguousarray((w_qkv[0:C] / 8.0).T)
    wk_t = np.ascontiguousarray(w_qkv[C:2 * C].T)
    wv_t = np.ascontiguousarray(w_qkv[2 * C:3 * C].T)
    wp_t = np.ascontiguousarray(w_proj.T)
    bias2 = np.ascontiguousarray(b_proj.reshape(2, 128).T)

    in_maps = []
    for core in range(8):
        b, half = divmod(core, 2)
        n0 = half * NQ
        x_rot = np.concatenate([x2[b][:, n0:], x2[b][:, :n0]], axis=1)
        in_maps.append({
            "x": np.ascontiguousarray(x_rot),
            "wq": wq_t, "wk": wk_t, "wv": wv_t, "wp": wp_t,
            "bias": bias2,
        })

    res = run_bass_kernel_spmd(nc, in_maps, list(range(8)))

    y = np.empty((B, C, N), np.float32)
    for core in range(8):
        b, half = divmod(core, 2)
        n0 = half * NQ
        y[b][:, n0:n0 + NQ] = res.results[core]["y"]
    return y.reshape(B, C, H, W)



# revision 5
# speedup vs baseline: 1.3223x; 1.3223x over previous
"""AttentionBlock (1x1-conv QKV + 4-head softmax attention + 1x1-conv proj)
on 8 Trainium2 NeuronCores.

Sharding: data-parallel over (batch b, query-half h) -> 8 shards. Each core
gets x rotated so its 2048 query columns are always columns 0:2048 (key order
is a permutation, which softmax-attention is invariant to), computes
qkv projections, 4-head attention for its half of the queries, and the output
projection for its [256, 2048] output slice. No collectives.

Core kernel tricks:
  - all matmuls in float32r (full-rate PE, ~1.5e-4 rel rounding); f32r is
    bit-identical to f32, so inputs are DMA'd straight into f32r tiles
    (no cast copies)
  - S^T = K^T Q with two heads row-tiled in the PE array (K=64 each)
  - softmax rowsums folded into the attn@V matmul: each head's V tile gets
    an extra ones column (M=65, tile_position (0,0)), so the rowsum
    accumulates at PSUM partition 64 of the same bank as the head's output
    (the four per-chunk ones-matmuls of the naive scheme disappear)
  - exp of scores: half the heads on the Scalar engine (exact), half via a
    fused custom DVE op ((x+c0)((x+c1)x+c2))^8 ~ C*e^x (scale cancels in
    softmax; assignment is per-(head, query-tile) so rows stay consistent)
  - normalization: rowsum rows -> SBUF, tiny K=1 matmuls broadcast each
    head's rowsum to partitions 0:64 (pairs packed in one borrowed PSUM
    tile), one reciprocal per pair, per-head multiplies (all base 0)
  - output projection per head (K=64 matmuls vs row-sliced w_proj),
    pipelined per query-tile into the attention loop
"""
import sys

sys.path.insert(0, '/opt/trn_rl_repo')

import numpy as np
from contextlib import ExitStack

from concourse import bass, bacc, mybir
import concourse.tile as tile
from concourse import dve_ops
from concourse.dve_ops import DveOp, OPS, CUSTOM_DVE_SPECS, _SUB_OPCODE_FOR_NAME
from concourse.dve_spec import Spec, Src0, Src1, C0, C1, C2, C3, lower, sq, _spill_c3_to_src1
from concourse.dve_uop import DveOpSpec
from concourse.bass_utils import run_bass_kernel_spmd

F32 = mybir.dt.float32
F32R = mybir.dt.float32r
BF16 = mybir.dt.bfloat16
ActFn = mybir.ActivationFunctionType

B, C, H, W = 4, 256, 64, 64
HEADS, DH = 4, 64
N = H * W            # 4096 keys
NQ = N // 2          # 2048 queries per core
NT = 512             # query tile (one PSUM bank of fp32)
N_NT = NQ // NT      # 4 query tiles
N_MC = N // 128      # 32 key chunks
VTW = 260            # vT cols per key chunk: [V_h(64) | 1] x 4 heads

# exp(x) ~ C * [q3(x) * (x^2 + b0 x + b1)]^16 over x in [-8.8, 8.4]
# (max rel err 3.3e-4; the constant C cancels in softmax normalization).
# Two DVE instructions: EXP5A computes the cubic q3, EXP5B multiplies by the
# monic quadratic and raises to the 16th power.
EXP_A = (0.00039684202121525346, 2.589769573122113e-05,
         6.891462469732395e-07, 7.771052073346383e-09)   # a0..a3
EXP_B = (-6.95331830849084, 2519.7822812996437)          # b0, b1


def _ref_exp5a(in0, in1, c0, c1, c2):
    x = in0.astype(np.float32)
    a3 = in1.astype(np.float32) if isinstance(in1, np.ndarray) else np.float32(in1)
    return (((a3 * x + np.float32(c2)) * x + np.float32(c1)) * x
            + np.float32(c0)).astype(np.float32)


def _ref_exp5b(in0, in1, c0, c1, c2):
    x = in0.astype(np.float32)
    q3 = in1.astype(np.float32)
    p = (q3 * ((x + np.float32(c0)) * x + np.float32(c1))).astype(np.float32)
    for _ in range(4):
        p = (p * p).astype(np.float32)
    return p


def _register(name, spec, rd1_en):
    row = dve_ops._CUSTOM_DVE_ROW_BASE + len(OPS)
    assert row < 0x20
    _SUB_OPCODE_FOR_NAME[name] = row
    shas = {}
    for ver in ("v3", "v4"):
        uops = lower(spec, ver=ver)
        shas[ver] = DveOpSpec(name=name, opcode=row, uops=uops, rd1_en=rd1_en).sha(ver)
    op = DveOp(name, spec, subdim=False, uops_sha=shas)
    OPS.append(op)
    CUSTOM_DVE_SPECS[name] = spec
    return op


def register_exp_op():
    if "EXP5A_ANT" in _SUB_OPCODE_FOR_NAME:
        a = next(op for op in OPS if op.name == "EXP5A_ANT")
        b = next(op for op in OPS if op.name == "EXP5B_ANT")
        return a, b
    x = Src0
    body_a = _spill_c3_to_src1(((C3 * x + C2) * x + C1) * x + C0)
    op_a = _register("EXP5A_ANT", Spec(body=body_a, reference=_ref_exp5a), True)
    body_b = sq(sq(sq(sq(Src1 * ((x + C0) * x + C1)))))
    op_b = _register("EXP5B_ANT", Spec(body=body_b, reference=_ref_exp5b), True)
    return op_a, op_b


def emit_exp_dve(nc, ops, out, in_, y1, a3_t):
    op_a, op_b = ops
    nc.vector._custom_dve(op_a, out=y1, in0=in_, in1=a3_t,
                          s0=float(EXP_A[0]), s1=float(EXP_A[1]), imm2=float(EXP_A[2]))
    return nc.vector._custom_dve(op_b, out=out, in0=in_, in1=y1,
                                 s0=float(EXP_B[0]), s1=float(EXP_B[1]))


# exp-engine split: ACT computes pair-0 tiles fully plus the first EXP_N0
# query-columns of each pair-1 head; the DVE two-op pipeline takes the rest.
# Constant per (pair, nt, n-range) so every softmax row uses one implementation.
import os as _os
EXP_N0 = int(_os.environ.get("EXP_N0", "224"))


def build_program(exp_op):
    nc = bacc.Bacc(target_bir_lowering=False)

    x_d = nc.declare_dram_parameter("x", [C, N], F32R, isOutput=False)
    wq_d = nc.declare_dram_parameter("wq", [C, C], F32R, isOutput=False)
    wk_d = nc.declare_dram_parameter("wk", [C, C], F32R, isOutput=False)
    wv_d = nc.declare_dram_parameter("wv", [C, C], F32R, isOutput=False)
    wp_d = nc.declare_dram_parameter("wp", [C, C], F32R, isOutput=False)
    bias_d = nc.declare_dram_parameter("bias", [128, 2], F32, isOutput=False)
    y_d = nc.declare_dram_parameter("y", [C, NQ], F32, isOutput=True)

    with tile.TileContext(nc) as tc, ExitStack() as ctx:
        sb = ctx.enter_context(tc.tile_pool(name="sb", bufs=1))
        pex = ctx.enter_context(tc.tile_pool(name="pex", bufs=3))
        pout = ctx.enter_context(tc.tile_pool(name="pout", bufs=2))
        ps = ctx.enter_context(tc.tile_pool(name="ps", bufs=1, space="PSUM"))

        # ---------------- load inputs (f32r == f32 bits; no casts) --------
        w_sb = {}
        for name, dram in (("wq", wq_d), ("wk", wk_d), ("wv", wv_d)):
            tiles = []
            for kc in range(2):
                r = sb.tile([128, C], F32R, tag=f"{name}{kc}", name=f"{name}r{kc}")
                nc.sync.dma_start(out=r, in_=dram[kc * 128:(kc + 1) * 128, :])
                tiles.append(r)
            w_sb[name] = tiles
        # w_proj row-sliced per head (64-row tiles at partition base 0) for
        # the per-head K=64 projection matmuls
        wph_sb = []
        for h in range(HEADS):
            t = sb.tile([64, C], F32R, tag=f"wph{h}", name=f"wph{h}")
            nc.sync.dma_start(out=t, in_=wp_d[h * 64:(h + 1) * 64, :])
            wph_sb.append(t)
        # x as 2 (channel-half) x 4 (col-quarter) tiles so compute can start
        # after the first quarter lands
        NXQ = N // 4
        x_r = [[sb.tile([128, NXQ], F32R, tag=f"x{kc}{qq}", name=f"x{kc}{qq}")
                for qq in range(4)] for kc in range(2)]
        for qq in range(4):
            for kc in range(2):
                nc.sync.dma_start(
                    out=x_r[kc][qq],
                    in_=x_d[kc * 128:(kc + 1) * 128, qq * NXQ:(qq + 1) * NXQ])
        bias_sb = sb.tile([128, 2], F32, tag="bias")
        nc.sync.dma_start(out=bias_sb, in_=bias_d[:, :])

        def xs(kc, col, width):
            """AP for x columns [col, col+width) in channel-half kc."""
            qq, lo = divmod(col, NXQ)
            assert lo + width <= NXQ
            return x_r[kc][qq][:, lo:lo + width]

        # constants
        a3_t = sb.tile([128, 1], F32, tag="a3")
        nc.vector.memset(a3_t, float(EXP_A[3]))
        # ones row at partition 64 for the K=1 denominator-broadcast matmuls
        bcone = sb.tile([65, 64], F32R, tag="bcone")
        nc.vector.memset(bcone[64:65, :].bitcast(F32), 1.0)

        # ---------------- phase 1: qkv projections ----------------
        q_sb = [sb.tile([128, NQ], F32R, tag=f"q{oc}", name=f"q_sb{oc}") for oc in range(2)]
        k_sb = [sb.tile([128, N], F32R, tag=f"k{oc}", name=f"k_sb{oc}") for oc in range(2)]
        vT_sb = sb.tile([128, N_MC * VTW], BF16, tag="vT")
        # ones columns at 65*j + 64 for all j (one per head per chunk)
        ap = vT_sb[:, 64:65]
        ap = bass.AP(tensor=ap.tensor, offset=ap.offset,
                     ap=[list(ap.ap[0]), [65, 4 * N_MC], [1, 1]])
        nc.vector.memset(ap, 1.0)

        for oc in range(2):
            for nt in range(N_NT):
                pq = ps.tile([128, 512], F32, tag=f"o{nt}", name=f"pq{oc}_{nt}")
                nc.tensor.matmul(out=pq[:, :], lhsT=w_sb["wq"][0][:, oc * 128:(oc + 1) * 128],
                                 rhs=xs(0, nt * 512, 512), start=True, stop=False)
                nc.tensor.matmul(out=pq[:, :], lhsT=w_sb["wq"][1][:, oc * 128:(oc + 1) * 128],
                                 rhs=xs(1, nt * 512, 512), start=False, stop=True)
                sl = slice(nt * 512, (nt + 1) * 512)
                (nc.scalar.copy if (oc + nt) % 2 else nc.vector.tensor_copy)(q_sb[oc][:, sl], pq[:, :])
        for oc in range(2):
            for nt in range(2 * N_NT):
                pk = ps.tile([128, 512], F32, tag=f"o{nt % 4}", name=f"pk{oc}_{nt}")
                nc.tensor.matmul(out=pk[:, :], lhsT=w_sb["wk"][0][:, oc * 128:(oc + 1) * 128],
                                 rhs=xs(0, nt * 512, 512), start=True, stop=False)
                nc.tensor.matmul(out=pk[:, :], lhsT=w_sb["wk"][1][:, oc * 128:(oc + 1) * 128],
                                 rhs=xs(1, nt * 512, 512), start=False, stop=True)
                sl = slice(nt * 512, (nt + 1) * 512)
                (nc.vector.tensor_copy if nt % 2 else nc.scalar.copy)(k_sb[oc][:, sl], pk[:, :])
        for mc in range(N_MC):
            pv = ps.tile([128, 256], F32, tag="s", bufs=2, name=f"pv{mc}")
            nc.tensor.matmul(out=pv[:, :], lhsT=xs(0, mc * 128, 128), rhs=w_sb["wv"][0][:, :],
                             start=True, stop=False)
            nc.tensor.matmul(out=pv[:, :], lhsT=xs(1, mc * 128, 128), rhs=w_sb["wv"][1][:, :],
                             start=False, stop=True)
            # strided copy into [V0|1|V1|1|V2|1|V3|1] layout (V blocks at stride 65)
            base = mc * VTW
            dst = vT_sb[:, base:base + 64]
            dst = bass.AP(tensor=dst.tensor, offset=dst.offset,
                          ap=[list(dst.ap[0]), [65, 4], [1, 64]])
            src = pv[:, 0:64]
            src = bass.AP(tensor=src.tensor, offset=src.offset,
                          ap=[list(src.ap[0]), [64, 4], [1, 64]])
            (nc.vector.tensor_copy if mc % 2 else nc.scalar.copy)(dst, src)

        # ---------------- phase 2: attention + per-tile projection --------
        for nt in range(N_NT):
            qsl = slice(nt * 512, (nt + 1) * 512)
            # per-head output accumulators: values at partitions 0:64,
            # softmax rowsum at partition 64
            po = [ps.tile([65, 512], F32, tag=f"o{h}", name=f"po{h}_{nt}")
                  for h in range(4)]
            for mc in range(N_MC):
                msl = slice(mc * 128, (mc + 1) * 128)
                exps = []
                for pair in (0, 1):
                    pst = ps.tile([128, 1024], F32, tag="s", bufs=2, name=f"pst{pair}_{nt}_{mc}")
                    # S^T: two heads row-tiled (dh=64 each)
                    nc.tensor.matmul(out=pst[:, 0:512],
                                     lhsT=k_sb[pair][0:64, msl], rhs=q_sb[pair][0:64, qsl],
                                     start=True, stop=True, tile_position=(0, 0))
                    nc.tensor.matmul(out=pst[:, 512:1024],
                                     lhsT=k_sb[pair][64:128, msl], rhs=q_sb[pair][64:128, qsl],
                                     start=True, stop=True, tile_position=(64, 0))
                    et = pex.tile([128, 1024], BF16, tag=f"e{pair}", name=f"et{pair}_{nt}_{mc}")
                    if pair == 0 or EXP_N0 >= 512:
                        nc.scalar.activation(et[:, :], pst[:, :], ActFn.Exp)
                    elif EXP_N0 == 0:
                        y1 = pex.tile([128, 1024], F32, tag="y1", name=f"y1_{pair}_{nt}_{mc}")
                        emit_exp_dve(nc, exp_op, et[:, :], pst[:, :], y1[:, :], a3_t[:, :])
                    else:
                        # strided APs covering (h2 cols [a:b]) u (h3 cols [512+a:512+b])
                        def _two(ap_t, a, b):
                            base = ap_t[:, a:b]
                            return bass.AP(tensor=base.tensor, offset=base.offset,
                                           ap=[list(base.ap[0]), [512, 2], [1, b - a]])
                        nc.scalar.activation(_two(et, 0, EXP_N0), _two(pst, 0, EXP_N0),
                                             ActFn.Exp)
                        y1 = pex.tile([128, 1024], F32, tag="y1", name=f"y1_{pair}_{nt}_{mc}")
                        emit_exp_dve(nc, exp_op, _two(et, EXP_N0, 512),
                                     _two(pst, EXP_N0, 512), _two(y1, EXP_N0, 512),
                                     a3_t[:, :])
                    exps.append(et)
                first, last = mc == 0, mc == N_MC - 1
                vb = mc * VTW
                for h in range(4):
                    nc.tensor.matmul(out=po[h][0:65, :],
                                     lhsT=vT_sb[:, vb + 65 * h:vb + 65 * h + 65],
                                     rhs=exps[h // 2][:, (h % 2) * 512:(h % 2 + 1) * 512],
                                     start=first, stop=last, tile_position=(0, 0))
            # normalization: rowsum rows -> SBUF (same partition), K=1 matmuls
            # broadcast each head's rowsum to partitions 0:64 (two heads packed
            # per borrowed PSUM tile), reciprocal per pair, per-head multiply.
            rs = [sb.tile([65, 512], F32R, tag=f"rs{h}", bufs=2, name=f"rs{h}_{nt}")
                  for h in range(4)]
            for h in range(4):
                nc.scalar.copy(rs[h][64:65, :], po[h][64:65, :])
            pbq = []
            for p_ in range(2):
                t = ps.tile([64, 1024], F32, tag="s", bufs=2, name=f"pbq{p_}_{nt}")
                for i in range(2):
                    nc.tensor.matmul(out=t[0:64, i * 512:(i + 1) * 512],
                                     lhsT=bcone[64:65, :], rhs=rs[2 * p_ + i][64:65, :],
                                     start=True, stop=True, tile_position=(64, 0))
                pbq.append(t)
            rbq = []
            for p_ in range(2):
                t = sb.tile([64, 1024], F32, tag=f"rbq{p_}", bufs=2, name=f"rbq{p_}_{nt}")
                nc.vector.reciprocal_approx_fast(out=t[:, :], in_=pbq[p_][:, :])
                rbq.append(t)
            outn = []
            for h in range(4):
                t = sb.tile([64, 512], F32R, tag=f"on{h}", bufs=2, name=f"on{h}_{nt}")
                nc.vector.tensor_tensor(
                    out=t[:, :], in0=po[h][0:64, :],
                    in1=rbq[h // 2][:, (h % 2) * 512:(h % 2 + 1) * 512],
                    op=mybir.AluOpType.mult)
                outn.append(t)
            # ---- output projection + bias for this query tile ----
            for oc in range(2):
                py = ps.tile([128, 512], F32, tag=f"o{2 * (nt % 2) + oc}", name=f"py{oc}_{nt}")
                for h in range(4):
                    nc.tensor.matmul(out=py[:, :],
                                     lhsT=wph_sb[h][:, oc * 128:(oc + 1) * 128],
                                     rhs=outn[h][:, :],
                                     start=(h == 0), stop=(h == 3), tile_position=(0, 0))
                y_sb = pout.tile([128, 512], F32, tag="y", name=f"y_sb{oc}_{nt}")
                nc.vector.tensor_scalar_add(y_sb[:, :], py[:, :], bias_sb[:, oc:oc + 1])
                nc.sync.dma_start(out=y_d[oc * 128:(oc + 1) * 128, qsl], in_=y_sb[:, :])

    nc.compile()
    return nc


_CACHE = {}


def _get_program():
    if "nc" not in _CACHE:
        op = register_exp_op()
        _CACHE["nc"] = build_program(op)
    return _CACHE["nc"]


def kernel(x, w_qkv, w_proj, b_proj):
    x = np.asarray(x, np.float32)
    w_qkv = np.asarray(w_qkv, np.float32)
    w_proj = np.asarray(w_proj, np.float32)
    b_proj = np.asarray(b_proj, np.float32)

    nc = _get_program()

    x2 = x.reshape(B, C, N)
    wq_t = np.ascontiguousarray((w_qkv[0:C] / 8.0).T)
    wk_t = np.ascontiguousarray(w_qkv[C:2 * C].T)
    wv_t = np.ascontiguousarray(w_qkv[2 * C:3 * C].T)
    wp_t = np.ascontiguousarray(w_proj.T)
    bias2 = np.ascontiguousarray(b_proj.reshape(2, 128).T)

    in_maps = []
    for core in range(8):
        b, half = divmod(core, 2)
        n0 = half * NQ
        x_rot = np.concatenate([x2[b][:, n0:], x2[b][:, :n0]], axis=1)
        in_maps.append({
            "x": np.ascontiguousarray(x_rot),
            "wq": wq_t, "wk": wk_t, "wv": wv_t, "wp": wp_t,
            "bias": bias2,
        })

    res = run_bass_kernel_spmd(nc, in_maps, list(range(8)))

    y = np.empty((B, C, N), np.float32)
    for core in range(8):
        b, half = divmod(core, 2)
        n0 = half * NQ
        y[b][:, n0:n0 + NQ] = res.results[core]["y"]
    return y.reshape(B, C, H, W)


# revision 8
# speedup vs baseline: 1.5187x; 1.1485x over previous
"""AttentionBlock (1x1-conv QKV + 4-head softmax attention + 1x1-conv proj)
on 8 Trainium2 NeuronCores.

Sharding: data-parallel over (batch b, query-half h) -> 8 shards. Each core
gets x rotated so its 2048 query columns are always columns 0:2048 (key order
is a permutation, which softmax-attention is invariant to), computes
qkv projections, 4-head attention for its half of the queries, and the output
projection for its [256, 2048] output slice. No collectives.

Core kernel tricks:
  - all matmuls in float32r (full-rate PE, ~1.5e-4 rel rounding); f32r is
    bit-identical to f32, so inputs are DMA'd straight into f32r tiles
    (no cast copies)
  - S^T = K^T Q with two heads row-tiled in the PE array (K=64 each)
  - softmax rowsums folded into the attn@V matmul: each head's V tile gets
    an extra ones column (M=65, tile_position (0,0)), so the rowsum
    accumulates at PSUM partition 64 of the same bank as the head's output
    (the four per-chunk ones-matmuls of the naive scheme disappear)
  - exp of scores: half the heads on the Scalar engine (exact), half via a
    fused custom DVE op ((x+c0)((x+c1)x+c2))^8 ~ C*e^x (scale cancels in
    softmax; assignment is per-(head, query-tile) so rows stay consistent)
  - normalization: rowsum rows -> SBUF, tiny K=1 matmuls broadcast each
    head's rowsum to partitions 0:64 (pairs packed in one borrowed PSUM
    tile), one reciprocal per pair, per-head multiplies (all base 0)
  - output projection per head (K=64 matmuls vs row-sliced w_proj),
    pipelined per query-tile into the attention loop
"""
import sys

sys.path.insert(0, '/opt/trn_rl_repo')

import numpy as np
from contextlib import ExitStack

from concourse import bass, bacc, mybir
import concourse.tile as tile
from concourse import dve_ops
from concourse.dve_ops import DveOp, OPS, CUSTOM_DVE_SPECS, _SUB_OPCODE_FOR_NAME
from concourse.dve_spec import Spec, Src0, Src1, C0, C1, C2, C3, lower, sq, _spill_c3_to_src1
from concourse.dve_uop import DveOpSpec
from concourse.bass_utils import run_bass_kernel_spmd

F32 = mybir.dt.float32
F32R = mybir.dt.float32r
BF16 = mybir.dt.bfloat16
ActFn = mybir.ActivationFunctionType

B, C, H, W = 4, 256, 64, 64
HEADS, DH = 4, 64
N = H * W            # 4096 keys
NQ = N // 2          # 2048 queries per core
NT = 512             # query tile (one PSUM bank of fp32)
N_NT = NQ // NT      # 4 query tiles
N_MC = N // 128      # 32 key chunks
VTW = 260            # vT cols per key chunk: [V_h(64) | 1] x 4 heads

# exp(x) ~ C * [q3(x) * (x^2 + b0 x + b1)]^16 over x in [-8.8, 8.4]
# (max rel err 3.3e-4; the constant C cancels in softmax normalization).
# Two DVE instructions: EXP5A computes the cubic q3, EXP5B multiplies by the
# monic quadratic and raises to the 16th power.
EXP_A = (0.00039684202121525346, 2.589769573122113e-05,
         6.891462469732395e-07, 7.771052073346383e-09)   # a0..a3
EXP_B = (-6.95331830849084, 2519.7822812996437)          # b0, b1


def _ref_exp5a(in0, in1, c0, c1, c2):
    x = in0.astype(np.float32)
    a3 = in1.astype(np.float32) if isinstance(in1, np.ndarray) else np.float32(in1)
    return (((a3 * x + np.float32(c2)) * x + np.float32(c1)) * x
            + np.float32(c0)).astype(np.float32)


def _ref_exp5b(in0, in1, c0, c1, c2):
    x = in0.astype(np.float32)
    q3 = in1.astype(np.float32)
    p = (q3 * ((x + np.float32(c0)) * x + np.float32(c1))).astype(np.float32)
    for _ in range(4):
        p = (p * p).astype(np.float32)
    return p


def _register(name, spec, rd1_en):
    row = dve_ops._CUSTOM_DVE_ROW_BASE + len(OPS)
    assert row < 0x20
    _SUB_OPCODE_FOR_NAME[name] = row
    shas = {}
    for ver in ("v3", "v4"):
        uops = lower(spec, ver=ver)
        shas[ver] = DveOpSpec(name=name, opcode=row, uops=uops, rd1_en=rd1_en).sha(ver)
    op = DveOp(name, spec, subdim=False, uops_sha=shas)
    OPS.append(op)
    CUSTOM_DVE_SPECS[name] = spec
    return op


def register_exp_op():
    if "EXP5A_ANT" in _SUB_OPCODE_FOR_NAME:
        a = next(op for op in OPS if op.name == "EXP5A_ANT")
        b = next(op for op in OPS if op.name == "EXP5B_ANT")
        return a, b
    x = Src0
    body_a = _spill_c3_to_src1(((C3 * x + C2) * x + C1) * x + C0)
    op_a = _register("EXP5A_ANT", Spec(body=body_a, reference=_ref_exp5a), True)
    body_b = sq(sq(sq(sq(Src1 * ((x + C0) * x + C1)))))
    op_b = _register("EXP5B_ANT", Spec(body=body_b, reference=_ref_exp5b), True)
    return op_a, op_b


def emit_exp_dve(nc, ops, out, in_, y1, a3_t):
    op_a, op_b = ops
    nc.vector._custom_dve(op_a, out=y1, in0=in_, in1=a3_t,
                          s0=float(EXP_A[0]), s1=float(EXP_A[1]), imm2=float(EXP_A[2]))
    return nc.vector._custom_dve(op_b, out=out, in0=in_, in1=y1,
                                 s0=float(EXP_B[0]), s1=float(EXP_B[1]))


# exp-engine split: ACT computes pair-0 tiles fully plus the first EXP_N0
# query-columns of each pair-1 head; the DVE two-op pipeline takes the rest.
# Constant per (pair, nt, n-range) so every softmax row uses one implementation.
import os as _os
EXP_N0 = int(_os.environ.get("EXP_N0", "320"))
# attn@V matmuls are software-pipelined this many key-chunks behind the
# S^T/exp of the same chunk, so the PE's in-order stream never head-of-line
# blocks on the exp latency
AV_LAG = int(_os.environ.get("AV_LAG", "2"))


def build_program(exp_op):
    nc = bacc.Bacc(target_bir_lowering=False)

    x_d = nc.declare_dram_parameter("x", [C, N], F32R, isOutput=False)
    wq_d = nc.declare_dram_parameter("wq", [C, C], F32R, isOutput=False)
    wk_d = nc.declare_dram_parameter("wk", [C, C], F32R, isOutput=False)
    wv_d = nc.declare_dram_parameter("wv", [C, C], F32R, isOutput=False)
    wp_d = nc.declare_dram_parameter("wp", [C, C], F32R, isOutput=False)
    bias_d = nc.declare_dram_parameter("bias", [128, 2], F32, isOutput=False)
    y_d = nc.declare_dram_parameter("y", [C, NQ], F32, isOutput=True)

    with tile.TileContext(nc) as tc, ExitStack() as ctx:
        sb = ctx.enter_context(tc.tile_pool(name="sb", bufs=1))
        pex = ctx.enter_context(tc.tile_pool(name="pex", bufs=3))
        pout = ctx.enter_context(tc.tile_pool(name="pout", bufs=2))
        ps = ctx.enter_context(tc.tile_pool(name="ps", bufs=1, space="PSUM"))

        # ---------------- load inputs (f32r == f32 bits; no casts) --------
        w_sb = {}
        for name, dram in (("wq", wq_d), ("wk", wk_d), ("wv", wv_d)):
            tiles = []
            for kc in range(2):
                r = sb.tile([128, C], F32R, tag=f"{name}{kc}", name=f"{name}r{kc}")
                nc.sync.dma_start(out=r, in_=dram[kc * 128:(kc + 1) * 128, :])
                tiles.append(r)
            w_sb[name] = tiles
        # w_proj row-sliced per head (64-row tiles at partition base 0) for
        # the per-head K=64 projection matmuls
        wph_sb = []
        for h in range(HEADS):
            t = sb.tile([64, C], F32R, tag=f"wph{h}", name=f"wph{h}")
            nc.sync.dma_start(out=t, in_=wp_d[h * 64:(h + 1) * 64, :])
            wph_sb.append(t)
        # x as 2 (channel-half) x 4 (col-quarter) tiles so compute can start
        # after the first quarter lands
        NXQ = N // 4
        x_r = [[sb.tile([128, NXQ], F32R, tag=f"x{kc}{qq}", name=f"x{kc}{qq}")
                for qq in range(4)] for kc in range(2)]
        for qq in range(4):
            for kc in range(2):
                nc.sync.dma_start(
                    out=x_r[kc][qq],
                    in_=x_d[kc * 128:(kc + 1) * 128, qq * NXQ:(qq + 1) * NXQ])
        bias_sb = sb.tile([128, 2], F32, tag="bias")
        nc.sync.dma_start(out=bias_sb, in_=bias_d[:, :])

        def xs(kc, col, width):
            """AP for x columns [col, col+width) in channel-half kc."""
            qq, lo = divmod(col, NXQ)
            assert lo + width <= NXQ
            return x_r[kc][qq][:, lo:lo + width]

        # constants
        a3_t = sb.tile([128, 1], F32, tag="a3")
        nc.vector.memset(a3_t, float(EXP_A[3]))
        # ones row at partition 64 for the K=1 denominator-broadcast matmuls
        bcone = sb.tile([65, 64], F32R, tag="bcone")
        nc.vector.memset(bcone[64:65, :].bitcast(F32), 1.0)

        # ---------------- phase 1: qkv projections ----------------
        q_sb = [sb.tile([128, NQ], F32R, tag=f"q{oc}", name=f"q_sb{oc}") for oc in range(2)]
        k_sb = [sb.tile([128, N], F32R, tag=f"k{oc}", name=f"k_sb{oc}") for oc in range(2)]
        vT_sb = sb.tile([128, N_MC * VTW], BF16, tag="vT")
        # ones columns at 65*j + 64 for all j (one per head per chunk)
        ap = vT_sb[:, 64:65]
        ap = bass.AP(tensor=ap.tensor, offset=ap.offset,
                     ap=[list(ap.ap[0]), [65, 4 * N_MC], [1, 1]])
        nc.vector.memset(ap, 1.0)

        for oc in range(2):
            for nt in range(N_NT):
                pq = ps.tile([128, 512], F32, tag=f"o{nt}", name=f"pq{oc}_{nt}")
                nc.tensor.matmul(out=pq[:, :], lhsT=w_sb["wq"][0][:, oc * 128:(oc + 1) * 128],
                                 rhs=xs(0, nt * 512, 512), start=True, stop=False)
                nc.tensor.matmul(out=pq[:, :], lhsT=w_sb["wq"][1][:, oc * 128:(oc + 1) * 128],
                                 rhs=xs(1, nt * 512, 512), start=False, stop=True)
                sl = slice(nt * 512, (nt + 1) * 512)
                (nc.scalar.copy if (oc + nt) % 2 else nc.vector.tensor_copy)(q_sb[oc][:, sl], pq[:, :])
        for oc in range(2):
            for nt in range(2 * N_NT):
                pk = ps.tile([128, 512], F32, tag=f"o{nt % 4}", name=f"pk{oc}_{nt}")
                nc.tensor.matmul(out=pk[:, :], lhsT=w_sb["wk"][0][:, oc * 128:(oc + 1) * 128],
                                 rhs=xs(0, nt * 512, 512), start=True, stop=False)
                nc.tensor.matmul(out=pk[:, :], lhsT=w_sb["wk"][1][:, oc * 128:(oc + 1) * 128],
                                 rhs=xs(1, nt * 512, 512), start=False, stop=True)
                sl = slice(nt * 512, (nt + 1) * 512)
                (nc.vector.tensor_copy if nt % 2 else nc.scalar.copy)(k_sb[oc][:, sl], pk[:, :])
        for mc in range(N_MC):
            pv = ps.tile([128, 256], F32, tag="s", bufs=2, name=f"pv{mc}")
            nc.tensor.matmul(out=pv[:, :], lhsT=xs(0, mc * 128, 128), rhs=w_sb["wv"][0][:, :],
                             start=True, stop=False)
            nc.tensor.matmul(out=pv[:, :], lhsT=xs(1, mc * 128, 128), rhs=w_sb["wv"][1][:, :],
                             start=False, stop=True)
            # strided copy into [V0|1|V1|1|V2|1|V3|1] layout (V blocks at stride 65)
            base = mc * VTW
            dst = vT_sb[:, base:base + 64]
            dst = bass.AP(tensor=dst.tensor, offset=dst.offset,
                          ap=[list(dst.ap[0]), [65, 4], [1, 64]])
            src = pv[:, 0:64]
            src = bass.AP(tensor=src.tensor, offset=src.offset,
                          ap=[list(src.ap[0]), [64, 4], [1, 64]])
            (nc.vector.tensor_copy if mc % 2 else nc.scalar.copy)(dst, src)

        # ---------------- phase 2: attention + per-tile projection --------
        for nt in range(N_NT):
            qsl = slice(nt * 512, (nt + 1) * 512)
            # per-head output accumulators: values at partitions 0:64,
            # softmax rowsum at partition 64
            po = [ps.tile([65, 512], F32, tag=f"o{h}", name=f"po{h}_{nt}")
                  for h in range(4)]
            ets = {}

            def emit_av(mc):
                first, last = mc == 0, mc == N_MC - 1
                vb = mc * VTW
                for h in range(4):
                    nc.tensor.matmul(out=po[h][0:65, :],
                                     lhsT=vT_sb[:, vb + 65 * h:vb + 65 * h + 65],
                                     rhs=ets[mc][h // 2][:, (h % 2) * 512:(h % 2 + 1) * 512],
                                     start=first, stop=last, tile_position=(0, 0))
                del ets[mc]

            for mc in range(N_MC):
                msl = slice(mc * 128, (mc + 1) * 128)
                exps = []
                for pair in (0, 1):
                    pst = ps.tile([128, 1024], F32, tag="s", bufs=2, name=f"pst{pair}_{nt}_{mc}")
                    # S^T: two heads row-tiled (dh=64 each)
                    nc.tensor.matmul(out=pst[:, 0:512],
                                     lhsT=k_sb[pair][0:64, msl], rhs=q_sb[pair][0:64, qsl],
                                     start=True, stop=True, tile_position=(0, 0))
                    nc.tensor.matmul(out=pst[:, 512:1024],
                                     lhsT=k_sb[pair][64:128, msl], rhs=q_sb[pair][64:128, qsl],
                                     start=True, stop=True, tile_position=(64, 0))
                    et = pex.tile([128, 1024], BF16, tag=f"e{pair}", bufs=3 + AV_LAG,
                                  name=f"et{pair}_{nt}_{mc}")
                    if pair == 0 or EXP_N0 >= 512:
                        nc.scalar.activation(et[:, :], pst[:, :], ActFn.Exp)
                    elif EXP_N0 == 0:
                        y1 = pex.tile([128, 1024], F32, tag="y1", name=f"y1_{pair}_{nt}_{mc}")
                        emit_exp_dve(nc, exp_op, et[:, :], pst[:, :], y1[:, :], a3_t[:, :])
                    else:
                        # strided APs covering (h2 cols [a:b]) u (h3 cols [512+a:512+b])
                        def _two(ap_t, a, b):
                            base = ap_t[:, a:b]
                            return bass.AP(tensor=base.tensor, offset=base.offset,
                                           ap=[list(base.ap[0]), [512, 2], [1, b - a]])
                        nc.scalar.activation(_two(et, 0, EXP_N0), _two(pst, 0, EXP_N0),
                                             ActFn.Exp)
                        y1 = pex.tile([128, 1024], F32, tag="y1", name=f"y1_{pair}_{nt}_{mc}")
                        emit_exp_dve(nc, exp_op, _two(et, EXP_N0, 512),
                                     _two(pst, EXP_N0, 512), _two(y1, EXP_N0, 512),
                                     a3_t[:, :])
                    exps.append(et)
                ets[mc] = exps
                if mc >= AV_LAG:
                    emit_av(mc - AV_LAG)
            for mc in range(N_MC - AV_LAG, N_MC):
                emit_av(mc)
            # normalization: rowsum rows -> SBUF (same partition), K=1 matmuls
            # broadcast each head's rowsum to partitions 0:64 (two heads packed
            # per borrowed PSUM tile), reciprocal per pair, per-head multiply.
            rs = [sb.tile([65, 512], F32R, tag=f"rs{h}", bufs=2, name=f"rs{h}_{nt}")
                  for h in range(4)]
            for h in range(4):
                nc.vector.tensor_copy(rs[h][64:65, :], po[h][64:65, :])
            pbq = []
            for p_ in range(2):
                t = ps.tile([64, 1024], F32, tag="s", bufs=2, name=f"pbq{p_}_{nt}")
                for i in range(2):
                    nc.tensor.matmul(out=t[0:64, i * 512:(i + 1) * 512],
                                     lhsT=bcone[64:65, :], rhs=rs[2 * p_ + i][64:65, :],
                                     start=True, stop=True, tile_position=(64, 0))
                pbq.append(t)
            rbq = []
            for p_ in range(2):
                t = sb.tile([64, 1024], F32, tag=f"rbq{p_}", bufs=2, name=f"rbq{p_}_{nt}")
                nc.vector.reciprocal_approx_fast(out=t[:, :], in_=pbq[p_][:, :])
                rbq.append(t)
            outn = []
            for h in range(4):
                t = sb.tile([64, 512], F32R, tag=f"on{h}", bufs=2, name=f"on{h}_{nt}")
                nc.vector.tensor_tensor(
                    out=t[:, :], in0=po[h][0:64, :],
                    in1=rbq[h // 2][:, (h % 2) * 512:(h % 2 + 1) * 512],
                    op=mybir.AluOpType.mult)
                outn.append(t)
            # ---- output projection + bias for this query tile ----
            for oc in range(2):
                py = ps.tile([128, 512], F32, tag=f"o{2 * (nt % 2) + oc}", name=f"py{oc}_{nt}")
                for h in range(4):
                    nc.tensor.matmul(out=py[:, :],
                                     lhsT=wph_sb[h][:, oc * 128:(oc + 1) * 128],
                                     rhs=outn[h][:, :],
                                     start=(h == 0), stop=(h == 3), tile_position=(0, 0))
                y_sb = pout.tile([128, 512], F32, tag="y", name=f"y_sb{oc}_{nt}")
                nc.vector.tensor_scalar_add(y_sb[:, :], py[:, :], bias_sb[:, oc:oc + 1])
                nc.sync.dma_start(out=y_d[oc * 128:(oc + 1) * 128, qsl], in_=y_sb[:, :])

    nc.compile()
    return nc


_CACHE = {}


def _get_program():
    if "nc" not in _CACHE:
        op = register_exp_op()
        _CACHE["nc"] = build_program(op)
    return _CACHE["nc"]


def kernel(x, w_qkv, w_proj, b_proj):
    x = np.asarray(x, np.float32)
    w_qkv = np.asarray(w_qkv, np.float32)
    w_proj = np.asarray(w_proj, np.float32)
    b_proj = np.asarray(b_proj, np.float32)

    nc = _get_program()

    x2 = x.reshape(B, C, N)
    wq_t = np.ascontiguousarray((w_qkv[0:C] / 8.0).T)
    wk_t = np.ascontiguousarray(w_qkv[C:2 * C].T)
    wv_t = np.ascontiguousarray(w_qkv[2 * C:3 * C].T)
    wp_t = np.ascontiguousarray(w_proj.T)
    bias2 = np.ascontiguousarray(b_proj.reshape(2, 128).T)

    in_maps = []
    for core in range(8):
        b, half = divmod(core, 2)
        n0 = half * NQ
        x_rot = np.concatenate([x2[b][:, n0:], x2[b][:, :n0]], axis=1)
        in_maps.append({
            "x": np.ascontiguousarray(x_rot),
            "wq": wq_t, "wk": wk_t, "wv": wv_t, "wp": wp_t,
            "bias": bias2,
        })

    res = run_bass_kernel_spmd(nc, in_maps, list(range(8)))

    y = np.empty((B, C, N), np.float32)
    for core in range(8):
        b, half = divmod(core, 2)
        n0 = half * NQ
        y[b][:, n0:n0 + NQ] = res.results[core]["y"]
    return y.reshape(B, C, H, W)


# revision 25
# speedup vs baseline: 1.8643x; 1.2276x over previous
"""AttentionBlock (1x1-conv QKV + 4-head softmax attention + 1x1-conv proj)
on 8 Trainium2 NeuronCores.

Sharding: data-parallel over (batch b, query-half h) -> 8 shards. Each core
gets x rotated so its 2048 query columns are always columns 0:2048 (key order
is a permutation, which softmax-attention is invariant to), computes
qkv projections, 4-head attention for its half of the queries, and the output
projection for its [256, 2048] output slice. No collectives.

Core kernel tricks:
  - all matmuls in float32r (full-rate PE, ~1.5e-4 rel rounding); f32r is
    bit-identical to f32, so inputs are DMA'd straight into f32r tiles
    (no cast copies)
  - S^T = K^T Q with two heads row-tiled in the PE array (K=64 each)
  - softmax rowsums folded into the attn@V matmul: each head's V tile gets
    an extra ones column (M=65, tile_position (0,0)), so the rowsum
    accumulates at PSUM partition 64 of the same bank as the head's output
    (the four per-chunk ones-matmuls of the naive scheme disappear)
  - exp of scores: half the heads on the Scalar engine (exact), half via a
    fused custom DVE op ((x+c0)((x+c1)x+c2))^8 ~ C*e^x (scale cancels in
    softmax; assignment is per-(head, query-tile) so rows stay consistent)
  - normalization: rowsum rows -> SBUF, tiny K=1 matmuls broadcast each
    head's rowsum to partitions 0:64 (pairs packed in one borrowed PSUM
    tile), one reciprocal per pair, per-head multiplies (all base 0)
  - output projection per head (K=64 matmuls vs row-sliced w_proj),
    pipelined per query-tile into the attention loop
"""
import sys

sys.path.insert(0, '/opt/trn_rl_repo')

import numpy as np
from contextlib import ExitStack

from concourse import bass, bacc, mybir
import concourse.tile as tile
from concourse import dve_ops
from concourse.dve_ops import DveOp, OPS, CUSTOM_DVE_SPECS, _SUB_OPCODE_FOR_NAME
from concourse.dve_spec import Spec, Src0, Src1, C0, C1, C2, C3, lower, sq, _spill_c3_to_src1
from concourse.dve_uop import DveOpSpec
from concourse.bass_utils import run_bass_kernel_spmd

F32 = mybir.dt.float32
F32R = mybir.dt.float32r
BF16 = mybir.dt.bfloat16
ActFn = mybir.ActivationFunctionType

B, C, H, W = 4, 256, 64, 64
HEADS, DH = 4, 64
N = H * W            # 4096 keys
NQ = N // 2          # 2048 queries per core
NT = 512             # query tile (one PSUM bank of fp32)
N_NT = NQ // NT      # 4 query tiles
N_MC = N // 128      # 32 key chunks
VTW = 260            # vT cols per key chunk: [V_h(64) | 1] x 4 heads

# exp(x) ~ C * [q3(x) * (x^2 + b0 x + b1)]^16 over x in [-8.8, 8.4]
# (max rel err 3.3e-4; the constant C cancels in softmax normalization).
# Two DVE instructions: EXP5A computes the cubic q3, EXP5B multiplies by the
# monic quadratic and raises to the 16th power.
EXP_A = (0.00039684202121525346, 2.589769573122113e-05,
         6.891462469732395e-07, 7.771052073346383e-09)   # a0..a3
EXP_B = (-6.95331830849084, 2519.7822812996437)          # b0, b1


def _ref_exp5a(in0, in1, c0, c1, c2):
    x = in0.astype(np.float32)
    a3 = in1.astype(np.float32) if isinstance(in1, np.ndarray) else np.float32(in1)
    return (((a3 * x + np.float32(c2)) * x + np.float32(c1)) * x
            + np.float32(c0)).astype(np.float32)


def _ref_exp5b(in0, in1, c0, c1, c2):
    x = in0.astype(np.float32)
    q3 = in1.astype(np.float32)
    p = (q3 * ((x + np.float32(c0)) * x + np.float32(c1))).astype(np.float32)
    for _ in range(4):
        p = (p * p).astype(np.float32)
    return p


def _register(name, spec, rd1_en):
    row = dve_ops._CUSTOM_DVE_ROW_BASE + len(OPS)
    assert row < 0x20
    _SUB_OPCODE_FOR_NAME[name] = row
    shas = {}
    for ver in ("v3", "v4"):
        uops = lower(spec, ver=ver)
        shas[ver] = DveOpSpec(name=name, opcode=row, uops=uops, rd1_en=rd1_en).sha(ver)
    op = DveOp(name, spec, subdim=False, uops_sha=shas)
    OPS.append(op)
    CUSTOM_DVE_SPECS[name] = spec
    return op


def register_exp_op():
    if "EXP5A_ANT" in _SUB_OPCODE_FOR_NAME:
        a = next(op for op in OPS if op.name == "EXP5A_ANT")
        b = next(op for op in OPS if op.name == "EXP5B_ANT")
        return a, b
    x = Src0
    body_a = _spill_c3_to_src1(((C3 * x + C2) * x + C1) * x + C0)
    op_a = _register("EXP5A_ANT", Spec(body=body_a, reference=_ref_exp5a), True)
    body_b = sq(sq(sq(sq(Src1 * ((x + C0) * x + C1)))))
    op_b = _register("EXP5B_ANT", Spec(body=body_b, reference=_ref_exp5b), True)
    return op_a, op_b


def emit_exp_dve(nc, ops, out, in_, y1, a3_t):
    op_a, op_b = ops
    nc.vector._custom_dve(op_a, out=y1, in0=in_, in1=a3_t,
                          s0=float(EXP_A[0]), s1=float(EXP_A[1]), imm2=float(EXP_A[2]))
    return nc.vector._custom_dve(op_b, out=out, in0=in_, in1=y1,
                                 s0=float(EXP_B[0]), s1=float(EXP_B[1]))


# exp-engine split: ACT computes pair-0 tiles fully plus pair-1 columns
# [0:EXP_SPLIT] (head 2 first); the DVE two-op pipeline takes the contiguous
# remainder [EXP_SPLIT:1024]. Contiguous ranges keep the byte intervals of
# the two writers disjoint, so no false WAW dependency serializes ACT and
# DVE. Constant per (head, query) so every softmax row uses one
# implementation.
import os as _os
EXP_SPLIT = int(_os.environ.get("EXP_SPLIT", "512"))
# attn@V matmuls are software-pipelined this many key-chunks behind the
# S^T/exp of the same chunk, so the PE's in-order stream never head-of-line
# blocks on the exp latency. Heads 2/3 lag further: their po banks double as
# the projection accumulators of the previous query tile, which are only
# released once the previous tile's bias-add has drained.
AV_LAG = int(_os.environ.get("AV_LAG", "2"))
AV_LAG23 = int(_os.environ.get("AV_LAG23", "5"))


def build_program(exp_op):
    nc = bacc.Bacc(target_bir_lowering=False)

    x_d = nc.declare_dram_parameter("x", [C, N], F32R, isOutput=False)
    wq_d = nc.declare_dram_parameter("wq", [C, C], F32R, isOutput=False)
    wk_d = nc.declare_dram_parameter("wk", [C, C], F32R, isOutput=False)
    wv_d = nc.declare_dram_parameter("wv", [C, C], F32R, isOutput=False)
    wp_d = nc.declare_dram_parameter("wp", [C, C], F32R, isOutput=False)
    bias_d = nc.declare_dram_parameter("bias", [128, 2], F32, isOutput=False)
    y_d = nc.declare_dram_parameter("y", [C, NQ], F32, isOutput=True)

    with tile.TileContext(nc) as tc, ExitStack() as ctx:
        sb = ctx.enter_context(tc.tile_pool(name="sb", bufs=1))
        pex = ctx.enter_context(tc.tile_pool(name="pex", bufs=3))
        pout = ctx.enter_context(tc.tile_pool(name="pout", bufs=2))
        ps = ctx.enter_context(tc.tile_pool(name="ps", bufs=1, space="PSUM"))

        # ---------------- load inputs (f32r == f32 bits; no casts) --------
        w_sb = {}
        for name, dram in (("wq", wq_d), ("wk", wk_d), ("wv", wv_d)):
            tiles = []
            for kc in range(2):
                r = sb.tile([128, C], F32R, tag=f"{name}{kc}", name=f"{name}r{kc}")
                nc.sync.dma_start(out=r, in_=dram[kc * 128:(kc + 1) * 128, :])
                tiles.append(r)
            w_sb[name] = tiles
        # w_proj row-sliced per head (64-row tiles at partition base 0) for
        # the per-head K=64 projection matmuls
        wph_sb = []
        for h in range(HEADS):
            t = sb.tile([64, C], F32R, tag=f"wph{h}", name=f"wph{h}")
            nc.sync.dma_start(out=t, in_=wp_d[h * 64:(h + 1) * 64, :])
            wph_sb.append(t)
        # x as 2 (channel-half) x 4 (col-quarter) tiles so compute can start
        # after the first quarter lands
        NXQ = N // 4
        x_r = [[sb.tile([128, NXQ], F32R, tag=f"x{kc}{qq}", name=f"x{kc}{qq}")
                for qq in range(4)] for kc in range(2)]
        for qq in range(4):
            for kc in range(2):
                nc.sync.dma_start(
                    out=x_r[kc][qq],
                    in_=x_d[kc * 128:(kc + 1) * 128, qq * NXQ:(qq + 1) * NXQ])
        bias_sb = sb.tile([128, 2], F32, tag="bias")
        nc.sync.dma_start(out=bias_sb, in_=bias_d[:, :])

        def xs(kc, col, width):
            """AP for x columns [col, col+width) in channel-half kc."""
            qq, lo = divmod(col, NXQ)
            assert lo + width <= NXQ
            return x_r[kc][qq][:, lo:lo + width]

        # constants
        a3_t = sb.tile([128, 1], F32, tag="a3")
        nc.vector.memset(a3_t, float(EXP_A[3]))
        # ones row at partition 64 for the K=1 denominator-broadcast matmuls
        bcone = sb.tile([65, 64], F32R, tag="bcone")
        nc.vector.memset(bcone[64:65, :].bitcast(F32), 1.0)

        # ---------------- phase 1: qkv projections ----------------
        q_sb = [sb.tile([128, NQ], F32R, tag=f"q{oc}", name=f"q_sb{oc}") for oc in range(2)]
        k_sb = [sb.tile([128, N], F32R, tag=f"k{oc}", name=f"k_sb{oc}") for oc in range(2)]
        vT_sb = sb.tile([128, N_MC * VTW], BF16, tag="vT")
        # ones columns at 65*j + 64 for all j (one per head per chunk)
        ap = vT_sb[:, 64:65]
        ap = bass.AP(tensor=ap.tensor, offset=ap.offset,
                     ap=[list(ap.ap[0]), [65, 4 * N_MC], [1, 1]])
        nc.vector.memset(ap, 1.0)

        for oc in range(2):
            for nt in range(N_NT):
                pq = ps.tile([128, 512], F32, tag=f"o{nt}", name=f"pq{oc}_{nt}")
                nc.tensor.matmul(out=pq[:, :], lhsT=w_sb["wq"][0][:, oc * 128:(oc + 1) * 128],
                                 rhs=xs(0, nt * 512, 512), start=True, stop=False)
                nc.tensor.matmul(out=pq[:, :], lhsT=w_sb["wq"][1][:, oc * 128:(oc + 1) * 128],
                                 rhs=xs(1, nt * 512, 512), start=False, stop=True)
                sl = slice(nt * 512, (nt + 1) * 512)
                (nc.scalar.copy if (oc + nt) % 2 else nc.vector.tensor_copy)(q_sb[oc][:, sl], pq[:, :])
        for oc in range(2):
            for nt in range(2 * N_NT):
                pk = ps.tile([128, 512], F32, tag=f"o{nt % 4}", name=f"pk{oc}_{nt}")
                nc.tensor.matmul(out=pk[:, :], lhsT=w_sb["wk"][0][:, oc * 128:(oc + 1) * 128],
                                 rhs=xs(0, nt * 512, 512), start=True, stop=False)
                nc.tensor.matmul(out=pk[:, :], lhsT=w_sb["wk"][1][:, oc * 128:(oc + 1) * 128],
                                 rhs=xs(1, nt * 512, 512), start=False, stop=True)
                sl = slice(nt * 512, (nt + 1) * 512)
                (nc.vector.tensor_copy if nt % 2 else nc.scalar.copy)(k_sb[oc][:, sl], pk[:, :])
        for mc in range(N_MC):
            pv = ps.tile([128, 256], F32, tag=("sa" if mc % 2 else "sb"), name=f"pv{mc}")
            nc.tensor.matmul(out=pv[:, :], lhsT=xs(0, mc * 128, 128), rhs=w_sb["wv"][0][:, :],
                             start=True, stop=False)
            nc.tensor.matmul(out=pv[:, :], lhsT=xs(1, mc * 128, 128), rhs=w_sb["wv"][1][:, :],
                             start=False, stop=True)
            # strided copy into [V0|1|V1|1|V2|1|V3|1] layout (V blocks at stride 65)
            base = mc * VTW
            dst = vT_sb[:, base:base + 64]
            dst = bass.AP(tensor=dst.tensor, offset=dst.offset,
                          ap=[list(dst.ap[0]), [65, 4], [1, 64]])
            src = pv[:, 0:64]
            src = bass.AP(tensor=src.tensor, offset=src.offset,
                          ap=[list(src.ap[0]), [64, 4], [1, 64]])
            (nc.vector.tensor_copy if mc % 2 else nc.scalar.copy)(dst, src)

        # ---------------- phase 2: attention + per-tile projection --------
        for nt in range(N_NT):
            qsl = slice(nt * 512, (nt + 1) * 512)
            # per-head output accumulators: values at partitions 0:64,
            # softmax rowsum at partition 64
            po = [ps.tile([65, 512], F32, tag=f"o{h}", name=f"po{h}_{nt}")
                  for h in range(4)]
            ets = {}

            def av_one(mc, h, et):
                first, last = mc == 0, mc == N_MC - 1
                vb = mc * VTW
                nc.tensor.matmul(out=po[h][0:65, :],
                                 lhsT=vT_sb[:, vb + 65 * h:vb + 65 * h + 65],
                                 rhs=et,
                                 start=first, stop=last, tile_position=(0, 0))

            def emit_av01(mc):
                av_one(mc, 0, ets[mc][0][:, 0:512])
                av_one(mc, 1, ets[mc][0][:, 512:1024])

            def emit_av23(mc):
                av_one(mc, 2, ets[mc][1][:, :])
                av_one(mc, 3, ets[mc][2][:, :])

            for mc in range(N_MC):
                msl = slice(mc * 128, (mc + 1) * 128)
                # pair 0: S^T two heads row-tiled into one 2-bank tile,
                # single reader (ACT exp). pair 1: one single-bank tile per
                # head so the ACT (head 2) and DVE (head 3) exps read
                # different tiles and never serialize on each other.
                pst0 = ps.tile([128, 1024], F32, tag="s", name=f"pst0_{nt}_{mc}")
                nc.tensor.matmul(out=pst0[:, 0:512],
                                 lhsT=k_sb[0][0:64, msl], rhs=q_sb[0][0:64, qsl],
                                 start=True, stop=True, tile_position=(0, 0))
                nc.tensor.matmul(out=pst0[:, 512:1024],
                                 lhsT=k_sb[0][64:128, msl], rhs=q_sb[0][64:128, qsl],
                                 start=True, stop=True, tile_position=(64, 0))
                pst1a = ps.tile([128, 512], F32, tag="sa", name=f"pst1a_{nt}_{mc}")
                nc.tensor.matmul(out=pst1a[:, :],
                                 lhsT=k_sb[1][0:64, msl], rhs=q_sb[1][0:64, qsl],
                                 start=True, stop=True, tile_position=(0, 0))
                et0 = pex.tile([128, 1024], BF16, tag="e0", bufs=3 + AV_LAG,
                               name=f"et0_{nt}_{mc}")
                nc.scalar.activation(et0[:, :], pst0[:, :], ActFn.Exp)
                et1a = pex.tile([128, 512], BF16, tag="e1a", bufs=3 + AV_LAG23,
                                name=f"et1a_{nt}_{mc}")
                nc.scalar.activation(et1a[:, :], pst1a[:, :], ActFn.Exp)
                et1b = pex.tile([128, 512], BF16, tag="e1b", bufs=3 + AV_LAG23,
                                name=f"et1b_{nt}_{mc}")
                pst1b = ps.tile([128, 512], F32, tag="sb", name=f"pst1b_{nt}_{mc}")
                nc.tensor.matmul(out=pst1b[:, :],
                                 lhsT=k_sb[1][64:128, msl], rhs=q_sb[1][64:128, qsl],
                                 start=True, stop=True, tile_position=(64, 0))
                y1 = pex.tile([128, 512], F32, tag="y1", name=f"y1_{nt}_{mc}")
                emit_exp_dve(nc, exp_op, et1b[:, :], pst1b[:, :], y1[:, :],
                             a3_t[:, :])
                ets[mc] = [et0, et1a, et1b]
                if mc >= AV_LAG:
                    emit_av01(mc - AV_LAG)
                if mc >= AV_LAG23:
                    emit_av23(mc - AV_LAG23)
            for mc in range(N_MC - AV_LAG, N_MC):
                emit_av01(mc)
            for mc in range(N_MC - AV_LAG23, N_MC):
                emit_av23(mc)
            # normalization: rowsum rows -> SBUF (same partition), K=1 matmuls
            # broadcast each head's rowsum to partitions 0:64 (in transiently
            # borrowed PSUM banks), reciprocal per head, per-head multiply.
            # The po values are evacuated to SBUF by ACT first so the po banks
            # are released for the next query tile without waiting on the
            # reciprocal chain.
            rs = [sb.tile([65, 512], F32R, tag=f"rs{h}", bufs=2, name=f"rs{h}_{nt}")
                  for h in range(4)]
            o_raw = []
            for h in range(4):
                nc.vector.tensor_copy(rs[h][64:65, :], po[h][64:65, :])
                t = sb.tile([64, 512], F32, tag=f"oraw{h}", bufs=2, name=f"oraw{h}_{nt}")
                nc.scalar.copy(t[:, :], po[h][0:64, :])
                o_raw.append(t)
            rb = []
            for h in range(4):
                pb = ps.tile([64, 512], F32, tag=f"o{h}", name=f"pb{h}_{nt}")
                nc.tensor.matmul(out=pb[0:64, :],
                                 lhsT=bcone[64:65, :], rhs=rs[h][64:65, :],
                                 start=True, stop=True, tile_position=(64, 0))
                t = sb.tile([64, 512], F32, tag=f"rb{h}", bufs=2, name=f"rb{h}_{nt}")
                nc.vector.reciprocal_approx_fast(out=t[:, :], in_=pb[:, :])
                rb.append(t)
            outn = []
            for h in range(4):
                t = sb.tile([64, 512], F32R, tag=f"on{h}", bufs=2, name=f"on{h}_{nt}")
                nc.gpsimd.tensor_tensor(
                    out=t[:, :], in0=o_raw[h][:, :], in1=rb[h][:, :],
                    op=mybir.AluOpType.mult)
                outn.append(t)
            # ---- output projection + bias for this query tile ----
            for oc in range(2):
                py = ps.tile([128, 512], F32, tag=f"o{2 + oc}", name=f"py{oc}_{nt}")
                for h in range(4):
                    nc.tensor.matmul(out=py[:, :],
                                     lhsT=wph_sb[h][:, oc * 128:(oc + 1) * 128],
                                     rhs=outn[h][:, :],
                                     start=(h == 0), stop=(h == 3), tile_position=(0, 0))
                y_sb = pout.tile([128, 512], F32, tag="y", name=f"y_sb{oc}_{nt}")
                nc.vector.tensor_scalar_add(y_sb[:, :], py[:, :], bias_sb[:, oc:oc + 1])
                nc.sync.dma_start(out=y_d[oc * 128:(oc + 1) * 128, qsl], in_=y_sb[:, :])

    nc.compile()
    return nc


_CACHE = {}


def _get_program():
    if "nc" not in _CACHE:
        op = register_exp_op()
        _CACHE["nc"] = build_program(op)
    return _CACHE["nc"]


def kernel(x, w_qkv, w_proj, b_proj):
    x = np.asarray(x, np.float32)
    w_qkv = np.asarray(w_qkv, np.float32)
    w_proj = np.asarray(w_proj, np.float32)
    b_proj = np.asarray(b_proj, np.float32)

    nc = _get_program()

    x2 = x.reshape(B, C, N)
    wq_t = np.ascontiguousarray((w_qkv[0:C] / 8.0).T)
    wk_t = np.ascontiguousarray(w_qkv[C:2 * C].T)
    wv_t = np.ascontiguousarray(w_qkv[2 * C:3 * C].T)
    wp_t = np.ascontiguousarray(w_proj.T)
    bias2 = np.ascontiguousarray(b_proj.reshape(2, 128).T)

    in_maps = []
    for core in range(8):
        b, half = divmod(core, 2)
        n0 = half * NQ
        x_rot = np.concatenate([x2[b][:, n0:], x2[b][:, :n0]], axis=1)
        in_maps.append({
            "x": np.ascontiguousarray(x_rot),
            "wq": wq_t, "wk": wk_t, "wv": wv_t, "wp": wp_t,
            "bias": bias2,
        })

    res = run_bass_kernel_spmd(nc, in_maps, list(range(8)))

    y = np.empty((B, C, N), np.float32)
    for core in range(8):
        b, half = divmod(core, 2)
        n0 = half * NQ
        y[b][:, n0:n0 + NQ] = res.results[core]["y"]
    return y.reshape(B, C, H, W)
